# revision 7
# baseline (speedup 1.0000x reference)
"""Trainium2 kernel for nn_ClauseFunction (segment_reduce):
C[b,g] = softor_s(softand_l(x[b, I_i[g,s,l]])), gamma=1e-3.

Strategy: shard over G (each of 8 cores handles 256 g-columns; x replicated).
Per core: gather 256*32*8 = 65536 rows of xT (one row = x[:,j] for all 64 b,
256 bytes f32) from DRAM via gpsimd.dma_gather. Descriptor generation is the
bottleneck (Q7 SWDGE, ~8.5ns/idx per core pair), so gather calls are spread
over all 4 SWDGE queues: queue q runs on Q7 core pair (2q, 2q+1), so 4 pairs
generate descriptors concurrently (~2.2ns/idx effective).

With gamma=1e-3 the soft reductions are within ~1e-3 of hard min/max
(measured rel err 1.4e-3 << 2e-2 gate), so compute is plain min over l then
max over s on DVE. Per-chunk partial maxes keep the post-gather tail short,
and the last chunks use smaller gather calls so all 4 queues stay busy to
the end.

Layout: gathered tile [128 part, slots, 64 b]; partition p holds g' in
{2p, 2p+1}; group c = gl*32+s (gl=g' parity, s); a call covers whole groups
(8 l-slots each) for every partition.
"""

import numpy as np

import concourse.bacc as bacc
import concourse.bass as bass
import concourse.tile as tile
from concourse import library_config, mybir
from concourse.bass_utils import run_bass_kernel_spmd

B, G, S, L = 64, 2048, 32, 8
NCORES = 8
GSH = G // NCORES  # 256 g' per core
NQ = 4  # SWDGE queues (each on its own Q7 core pair)
GRP_PER_PART = GSH // 128 * S  # 64 groups (gl, s) per partition
# chunk schedule per gl-half: (idxs per call, calls per chunk); 4 calls per
# chunk, one per queue. A call of 1024 idxs covers 1 group (8 l-slots x 128
# partitions) and emits 64 descriptors/lane = the single_packet SDMA packet
# ceiling, so calls never exceed 1024 idxs. The per-chunk partial maxes keep
# the post-gather tail short without tapering.
HALF_SCHED = [(1024, 4)] * 8  # 32 groups
SCHED = HALF_SCHED * 2
NCHUNK = len(SCHED)

_nc_cache = None
last_result = None


def _v(t, dims, off=0):
    """View of tile t with explicit free-dim [stride, count] pairs (elements).

    Keeps the tile's own partition entry (stride = per-partition size)."""
    return bass.AP(tensor=t.tensor, offset=t.offset + off, ap=[list(t.ap[0])] + dims)


def _idx_cols():
    return sum(n * k for n, k in SCHED) // 16


def _build_nc():
    f32 = mybir.dt.float32
    nc = bacc.Bacc("TRN2", target_bir_lowering=False, num_swdge_queues=NQ)
    tbl_in = nc.dram_tensor("tbl", [G, B], f32, kind="ExternalInput")  # x.T
    idx_in = nc.dram_tensor("idx", [128, _idx_cols()], mybir.dt.int16, kind="ExternalInput")
    c_out = nc.dram_tensor("c", [128, 128], f32, kind="ExternalOutput")

    with tile.TileContext(nc) as tc:
        with (
            tc.tile_pool(name="singles", bufs=1) as singles,
            tc.tile_pool(name="gath", bufs=3) as gath,
            tc.tile_pool(name="small", bufs=2) as small,
        ):
            # start the ~11us gather-ucode IRAM load as early as possible
            nc.gpsimd.load_library(library_config.mlp)
            idxs = singles.tile([128, _idx_cols()], mybir.dt.int16)
            # split the idx load so the first chunk's gathers start early
            first_cols = SCHED[0][0] * SCHED[0][1] // 16
            nc.sync.dma_start(out=idxs[:, :first_cols], in_=idx_in[:, :first_cols])
            nc.sync.dma_start(out=idxs[:, first_cols:], in_=idx_in[:, first_cols:])
            vv = singles.tile([128, GRP_PER_PART, B], f32)  # per-group min_l
            pm = singles.tile([128, NCHUNK, B], f32)  # per-chunk partial max_s
            coff = 0  # idx column offset
            gbase = 0  # group offset
            call = 0
            for ch, (nidx, ncalls) in enumerate(SCHED):
                gpc = nidx // 1024  # groups per call
                cg = gpc * ncalls  # groups per chunk
                gt = gath.tile([128, cg * 8, B], f32, tag="gt")
                for ci in range(ncalls):
                    nc.gpsimd.dma_gather(
                        gt[:, ci * gpc * 8 : (ci + 1) * gpc * 8, :],
                        tbl_in[:, :],
                        idxs[:, coff : coff + nidx // 16],
                        num_idxs=nidx,
                        num_idxs_reg=nidx,
                        elem_size=B,
                        queue_num=call % NQ,
                    )
                    coff += nidx // 16
                    call += 1
                # gt slots = (grp cg, l 8), b innermost
                nc.vector.tensor_reduce(
                    out=vv[:, gbase : gbase + cg, :],
                    in_=_v(gt, [[8 * B, cg], [1, B], [B, 8]]),  # [grp, b, l]
                    axis=mybir.AxisListType.X,
                    op=mybir.AluOpType.min,
                )
                nc.vector.tensor_reduce(
                    out=pm[:, ch, :],
                    in_=_v(vv, [[1, B], [B, cg]], gbase * B),  # [b, grp]
                    axis=mybir.AxisListType.X,
                    op=mybir.AluOpType.max,
                )
                gbase += cg
                if gbase % 32 == 0:  # finished a gl half
                    gl = gbase // 32 - 1
                    nch = len(HALF_SCHED)
                    vm = small.tile([128, B], f32, tag="vm")
                    nc.vector.tensor_reduce(
                        out=vm,
                        in_=_v(pm, [[1, B], [B, nch]], gl * nch * B),  # [b, chunk]
                        axis=mybir.AxisListType.X,
                        op=mybir.AluOpType.max,
                    )
                    nc.sync.dma_start(out=c_out[:, gl * B : (gl + 1) * B], in_=vm)
    nc.finalize()
    return nc


def _prep_inputs(x: np.ndarray, I_i: np.ndarray):
    """Host-side layout: x transposed; per-core wrapped idx tensors."""
    tbl = np.ascontiguousarray(x.astype(np.float32, copy=False).T)  # [G, B]
    idx_maps = []
    I = np.asarray(I_i)
    for k in range(NCORES):
        Ik = I[k * GSH : (k + 1) * GSH]  # [256, 32, 8] values in [0, G)
        # group c = gl*32 + s ; partition p holds g' = 2p + gl
        Ikr = Ik.reshape(128, 2, S, L)  # [p, gl, s, l]
        lc = np.transpose(Ikr, (1, 2, 3, 0)).reshape(2 * S, L, 128)  # [c, l, p]
        parts = []
        gbase = 0
        for nidx, ncalls in SCHED:
            gpc = nidx // 1024
            for ci in range(ncalls):
                # call covers groups [gbase, gbase+gpc); list pos j = i*128+p,
                # i = gi*8 + l
                flat = lc[gbase : gbase + gpc].reshape(nidx)  # [gi, l, p] flat
                # wrapped: partition q slot t holds flat[t*16 + q%16]
                w = flat.reshape(nidx // 16, 16).T  # [16, nidx/16]
                parts.append(w)
                gbase += gpc
        wall = np.concatenate(parts, axis=1)  # [16, total_cols]
        idx = np.tile(wall, (8, 1)).astype(np.int16)  # replicate to 128 parts
        idx_maps.append(idx)
    return tbl, idx_maps


def kernel(x: np.ndarray, I_i: np.ndarray) -> np.ndarray:
    global _nc_cache, last_result
    if _nc_cache is None:
        _nc_cache = _build_nc()
    nc = _nc_cache
    tbl, idx_maps = _prep_inputs(x, I_i)
    in_maps = [{"tbl": tbl, "idx": idx_maps[k]} for k in range(NCORES)]
    res = run_bass_kernel_spmd(nc, in_maps, core_ids=list(range(NCORES)))
    last_result = res
    C = np.empty((B, G), dtype=np.float32)
    for k in range(NCORES):
        o = res.results[k]["c"].reshape(128, 2, B)  # [p, gl, b]
        C[:, k * GSH : (k + 1) * GSH] = np.transpose(o, (2, 0, 1)).reshape(B, GSH)
    return C


# revision 10
# speedup vs baseline: 1.0330x; 1.0330x over previous
"""Trainium2 kernel for nn_ClauseFunction (segment_reduce):
C[b,g] = softor_s(softand_l(x[b, I_i[g,s,l]])), gamma=1e-3.

Strategy: shard over G (each of 8 cores handles 256 g-columns; x replicated).
Per core: gather 256*32*8 = 65536 rows of xT (one row = x[:,j] for all 64 b,
256 bytes f32) from DRAM via gpsimd.dma_gather. Descriptor generation is the
bottleneck (Q7 SWDGE, ~8.5ns/idx per core pair), so gather calls are spread
over all 4 SWDGE queues: queue q runs on Q7 core pair (2q, 2q+1), so 4 pairs
generate descriptors concurrently (~2.2ns/idx effective).

With gamma=1e-3 the soft reductions are within ~1e-3 of hard min/max
(measured rel err 1.4e-3 << 2e-2 gate), so compute is plain min over l then
max over s on DVE. Per-chunk partial maxes keep the post-gather tail short,
and the last chunks use smaller gather calls so all 4 queues stay busy to
the end.

Layout: gathered tile [128 part, slots, 64 b]; partition p holds g' in
{2p, 2p+1}; group c = gl*32+s (gl=g' parity, s); a call covers whole groups
(8 l-slots each) for every partition.
"""

import numpy as np

import concourse.bacc as bacc
import concourse.bass as bass
import concourse.tile as tile
from concourse import library_config, mybir
from concourse.bass_utils import run_bass_kernel_spmd

B, G, S, L = 64, 2048, 32, 8
NCORES = 8
GSH = G // NCORES  # 256 g' per core
NQ = 4  # SWDGE queues (each on its own Q7 core pair)
GRP_PER_PART = GSH // 128 * S  # 64 groups (gl, s) per partition
# chunk schedule per gl-half: (idxs per call, calls per chunk); 4 calls per
# chunk, one per queue. A call of n idxs covers n/1024 groups (1024 idx =
# 8 l-slots x 128 partitions). Calls over 1024 idxs exceed the 64-descriptor
# single_packet ceiling, so gathers run with single_packet=False (one packet
# per descriptor; SDMA has slack for the extra per-packet overhead). Tapered
# so the final compute tail stays short.
HALF_SCHED = [(2048, 4)] * 3 + [(1024, 4)] * 2  # 24 + 8 = 32 groups
SCHED = HALF_SCHED * 2
NCHUNK = len(SCHED)

_nc_cache = None
last_result = None


def _v(t, dims, off=0):
    """View of tile t with explicit free-dim [stride, count] pairs (elements).

    Keeps the tile's own partition entry (stride = per-partition size)."""
    return bass.AP(tensor=t.tensor, offset=t.offset + off, ap=[list(t.ap[0])] + dims)


def _idx_cols():
    return sum(n * k for n, k in SCHED) // 16


def _build_nc():
    f32 = mybir.dt.float32
    nc = bacc.Bacc(
        "TRN2",
        target_bir_lowering=False,
        num_swdge_queues=NQ,
        # 2048-idx gathers emit 129 descriptors/lane; the default 16KB scratch
        # gives 128-descriptor rings, so double it.
        dynamic_dma_scratch_size=32768,
    )
    tbl_in = nc.dram_tensor("tbl", [G, B], f32, kind="ExternalInput")  # x.T
    idx_in = nc.dram_tensor("idx", [128, _idx_cols()], mybir.dt.int16, kind="ExternalInput")
    c_out = nc.dram_tensor("c", [128, 128], f32, kind="ExternalOutput")

    with tile.TileContext(nc) as tc:
        with (
            tc.tile_pool(name="singles", bufs=1) as singles,
            tc.tile_pool(name="gath", bufs=3) as gath,
            tc.tile_pool(name="small", bufs=2) as small,
        ):
            # start the ~11us gather-ucode IRAM load as early as possible
            nc.gpsimd.load_library(library_config.mlp)
            idxs = singles.tile([128, _idx_cols()], mybir.dt.int16)
            # split the idx load so the first chunk's gathers start early
            first_cols = SCHED[0][0] * SCHED[0][1] // 16
            nc.sync.dma_start(out=idxs[:, :first_cols], in_=idx_in[:, :first_cols])
            nc.sync.dma_start(out=idxs[:, first_cols:], in_=idx_in[:, first_cols:])
            vv = singles.tile([128, GRP_PER_PART, B], f32)  # per-group min_l
            pm = singles.tile([128, NCHUNK, B], f32)  # per-chunk partial max_s
            coff = 0  # idx column offset
            gbase = 0  # group offset
            call = 0
            for ch, (nidx, ncalls) in enumerate(SCHED):
                gpc = nidx // 1024  # groups per call
                cg = gpc * ncalls  # groups per chunk
                gt = gath.tile([128, cg * 8, B], f32, tag="gt")
                for ci in range(ncalls):
                    nc.gpsimd.dma_gather(
                        gt[:, ci * gpc * 8 : (ci + 1) * gpc * 8, :],
                        tbl_in[:, :],
                        idxs[:, coff : coff + nidx // 16],
                        num_idxs=nidx,
                        num_idxs_reg=nidx,
                        elem_size=B,
                        queue_num=call % NQ,
                        single_packet=False,
                    )
                    coff += nidx // 16
                    call += 1
                # gt slots = (grp cg, l 8), b innermost
                nc.vector.tensor_reduce(
                    out=vv[:, gbase : gbase + cg, :],
                    in_=_v(gt, [[8 * B, cg], [1, B], [B, 8]]),  # [grp, b, l]
                    axis=mybir.AxisListType.X,
                    op=mybir.AluOpType.min,
                )
                nc.vector.tensor_reduce(
                    out=pm[:, ch, :],
                    in_=_v(vv, [[1, B], [B, cg]], gbase * B),  # [b, grp]
                    axis=mybir.AxisListType.X,
                    op=mybir.AluOpType.max,
                )
                gbase += cg
                if gbase % 32 == 0:  # finished a gl half
                    gl = gbase // 32 - 1
                    nch = len(HALF_SCHED)
                    vm = small.tile([128, B], f32, tag="vm")
                    nc.vector.tensor_reduce(
                        out=vm,
                        in_=_v(pm, [[1, B], [B, nch]], gl * nch * B),  # [b, chunk]
                        axis=mybir.AxisListType.X,
                        op=mybir.AluOpType.max,
                    )
                    nc.sync.dma_start(out=c_out[:, gl * B : (gl + 1) * B], in_=vm)
    nc.finalize()
    return nc


def _prep_inputs(x: np.ndarray, I_i: np.ndarray):
    """Host-side layout: x transposed; per-core wrapped idx tensors."""
    tbl = np.ascontiguousarray(x.astype(np.float32, copy=False).T)  # [G, B]
    idx_maps = []
    I = np.asarray(I_i)
    for k in range(NCORES):
        Ik = I[k * GSH : (k + 1) * GSH]  # [256, 32, 8] values in [0, G)
        # group c = gl*32 + s ; partition p holds g' = 2p + gl
        Ikr = Ik.reshape(128, 2, S, L)  # [p, gl, s, l]
        lc = np.transpose(Ikr, (1, 2, 3, 0)).reshape(2 * S, L, 128)  # [c, l, p]
        parts = []
        gbase = 0
        for nidx, ncalls in SCHED:
            gpc = nidx // 1024
            for ci in range(ncalls):
                # call covers groups [gbase, gbase+gpc); list pos j = i*128+p,
                # i = gi*8 + l
                flat = lc[gbase : gbase + gpc].reshape(nidx)  # [gi, l, p] flat
                # wrapped: partition q slot t holds flat[t*16 + q%16]
                w = flat.reshape(nidx // 16, 16).T  # [16, nidx/16]
                parts.append(w)
                gbase += gpc
        wall = np.concatenate(parts, axis=1)  # [16, total_cols]
        idx = np.tile(wall, (8, 1)).astype(np.int16)  # replicate to 128 parts
        idx_maps.append(idx)
    return tbl, idx_maps


def kernel(x: np.ndarray, I_i: np.ndarray) -> np.ndarray:
    global _nc_cache, last_result
    if _nc_cache is None:
        _nc_cache = _build_nc()
    nc = _nc_cache
    tbl, idx_maps = _prep_inputs(x, I_i)
    in_maps = [{"tbl": tbl, "idx": idx_maps[k]} for k in range(NCORES)]
    res = run_bass_kernel_spmd(nc, in_maps, core_ids=list(range(NCORES)))
    last_result = res
    C = np.empty((B, G), dtype=np.float32)
    for k in range(NCORES):
        o = res.results[k]["c"].reshape(128, 2, B)  # [p, gl, b]
        C[:, k * GSH : (k + 1) * GSH] = np.transpose(o, (2, 0, 1)).reshape(B, GSH)
    return C
